# revision 2
# baseline (speedup 1.0000x reference)
"""Trainium2 Bass kernel for GNN message passing (SSIM-weighted edge aggregation).

Problem (per batch element, one NeuronCore each; B=8 across 8 cores):
  x, xp: [C=96, N=3136];  edge_index: idx_i/idx_j [N, K=18] node ids.
  For each (n, k): gather channel columns x_i = x[:, idx_i], x_j = x[:, idx_j],
  compute SSIM-like scalar sff(n,k) from channel stats, output
    Ex[c, n] = sum_k |xp_i - xp_j| * sff + sum_k xp_i + sum_k xp_j.

Device strategy (v2 — bf16 table, host-precomputed stats):
  1. Host builds DRAM table [N, 256] bf16 rows =
     [x~.T(96) | xp.T(96) | mu | mu^2 | var | pad], where x~ = x - mu
     (mean-centered per node over channels, so cov = dot(x~_i, x~_j)/96).
  2. Per chunk of 128 nodes: one SWDGE dma_gather of 4608 512B rows
     (i-side then j-side pairs; pair g lands on partition g%128, block g//128).
  3. DVE (bf16 2x/4x modes): pair products + K-reduce for dots, |dxp| via
     int16 bitwise-and abs, per-pair sff scalars from gathered stats.
  4. K-reduction via TensorE in bf16 (1 cyc/row): block one-hot matrices M
     as matmul weights; sff folded into M for the |dxp| term. PSUM
     accumulates 54 matmuls -> [128 nodes, 96 ch], DMA to DRAM [N, C] f32.
Host reassembles: out.T per core, stack -> [8, 96, 3136, 1].
"""

import sys

import numpy as np

sys.path.insert(0, "/opt/trn_rl_repo")

import ml_dtypes  # noqa: E402

B, C, N, K = 8, 96, 3136, 18
C1 = 1e-6
C2 = 1e-6
ROW = 256  # table row bf16 elements (512B; dma_gather needs %256B==0)
NCH = (N + 127) // 128  # 25 chunks (24 x 128 nodes + 1 x 64)
MAXCOLS = 2 * 128 * K // 16  # idx cols per chunk (288)

_nc_cache = None


def _build_nc():
    import concourse.bacc as bacc
    import concourse.mybir as mybir
    import concourse.tile as tile
    from concourse.library_config import mlp
    from contextlib import ExitStack

    f32 = mybir.dt.float32
    bf16 = mybir.dt.bfloat16
    i16 = mybir.dt.int16
    AF = mybir.ActivationFunctionType
    OP = mybir.AluOpType
    AX = mybir.AxisListType

    nc = bacc.Bacc(None, target_bir_lowering=False, debug=False)

    table_d = nc.dram_tensor("table", [N, ROW], bf16, kind="ExternalInput")
    idx_d = nc.dram_tensor("idx", [128, NCH, MAXCOLS], i16, kind="ExternalInput")
    mb_d = nc.dram_tensor("mb", [128, 2 * K, 64], bf16, kind="ExternalInput")
    out_d = nc.dram_tensor("out", [N, C], f32, kind="ExternalOutput")

    with ExitStack() as ctx:
        tc = ctx.enter_context(tile.TileContext(nc))
        const = ctx.enter_context(tc.tile_pool(name="const", bufs=1))
        gath = ctx.enter_context(tc.tile_pool(name="gath", bufs=2))
        work = ctx.enter_context(tc.tile_pool(name="work", bufs=2))
        stat = ctx.enter_context(tc.tile_pool(name="stat", bufs=2))
        mpool = ctx.enter_context(tc.tile_pool(name="mpool", bufs=2))
        outp = ctx.enter_context(tc.tile_pool(name="outp", bufs=3))
        psum = ctx.enter_context(tc.tile_pool(name="psum", bufs=2, space="PSUM"))

        nc.gpsimd.load_library(mlp)

        idx_sb = const.tile([128, NCH, MAXCOLS], i16)
        nc.sync.dma_start(out=idx_sb[:], in_=idx_d[:])
        mb_sb = const.tile([128, 2 * K, 64], bf16)
        nc.sync.dma_start(out=mb_sb[:], in_=mb_d[:])

        for c in range(NCH):
            n0 = c * 128
            nr = min(128, N - n0)
            L = nr * K  # pairs per side
            nb = L // 128  # blocks per side (18 or 9)
            ni = 2 * L  # gathered rows

            gt = gath.tile([128, 2 * nb, ROW], bf16, tag="gt")
            nc.gpsimd.dma_gather(
                gt[:], table_d[:], idx_sb[:, c, 0 : ni // 16], ni, ni, ROW,
                single_packet=False,
            )

            x_i = gt[:, 0:nb, 0:96]
            x_j = gt[:, nb : 2 * nb, 0:96]
            xp_i = gt[:, 0:nb, 96:192]
            xp_j = gt[:, nb : 2 * nb, 96:192]
            mu_i = gt[:, 0:nb, 192]
            mu_j = gt[:, nb : 2 * nb, 192]
            mu2_i = gt[:, 0:nb, 193]
            mu2_j = gt[:, nb : 2 * nb, 193]
            var_i = gt[:, 0:nb, 194]
            var_j = gt[:, nb : 2 * nb, 194]

            # dots: P = x~_i * x~_j (bf16 2x), K-reduce -> ps [128, nb] f32
            P = work.tile([128, K, 96], bf16, tag="P")
            nc.vector.tensor_mul(out=P[:, :nb, :], in0=x_i, in1=x_j)
            ps = stat.tile([128, K], f32, tag="ps")
            nc.vector.tensor_reduce(
                out=ps[:, :nb], in_=P[:, :nb, :], axis=AX.X, op=OP.add
            )

            # A = |xp_i - xp_j| (bf16; abs = clear sign bit via int16 view)
            D = work.tile([128, K, 96], bf16, tag="D")
            nc.vector.tensor_sub(out=D[:, :nb, :], in0=xp_i, in1=xp_j)
            A = work.tile([128, K, 96], bf16, tag="A")
            nc.vector.tensor_scalar(
                out=A[:, :nb, :].bitcast(i16),
                in0=D[:, :nb, :].bitcast(i16),
                scalar1=0x7FFF,
                scalar2=None,
                op0=OP.bitwise_and,
            )

            def st(tag):
                return stat.tile([128, K], f32, tag=tag, name=f"{tag}_{c}")

            # sff = 1 - S1*S2;  S1 = (2 mu_i mu_j + C1)/(mu_i^2 + mu_j^2 + C1)
            #                   S2 = (2 cov + C2)/(var_i + var_j + C2)
            den2, r2, num2, S2 = st("den2"), st("r2"), st("num2"), st("S2")
            nc.vector.scalar_tensor_tensor(
                out=den2[:, :nb], in0=var_i, scalar=C2, in1=var_j,
                op0=OP.add, op1=OP.add,
            )
            nc.vector.reciprocal(out=r2[:, :nb], in_=den2[:, :nb])
            nc.vector.tensor_scalar(
                out=num2[:, :nb], in0=ps[:, :nb], scalar1=2.0 / 96.0, scalar2=C2,
                op0=OP.mult, op1=OP.add,
            )
            nc.vector.tensor_mul(out=S2[:, :nb], in0=num2[:, :nb], in1=r2[:, :nb])

            den1, r1, t1, t2, S1 = st("den1"), st("r1"), st("t1"), st("t2"), st("S1")
            nc.vector.scalar_tensor_tensor(
                out=den1[:, :nb], in0=mu2_i, scalar=C1, in1=mu2_j,
                op0=OP.add, op1=OP.add,
            )
            nc.vector.reciprocal(out=r1[:, :nb], in_=den1[:, :nb])
            nc.vector.tensor_mul(out=t1[:, :nb], in0=mu_i, in1=mu_j)
            nc.vector.tensor_scalar(
                out=t2[:, :nb], in0=t1[:, :nb], scalar1=2.0, scalar2=C1,
                op0=OP.mult, op1=OP.add,
            )
            nc.vector.tensor_mul(out=S1[:, :nb], in0=t2[:, :nb], in1=r1[:, :nb])

            s12 = st("s12")
            nc.vector.tensor_mul(out=s12[:, :nb], in0=S1[:, :nb], in1=S2[:, :nb])
            sffb = stat.tile([128, K], bf16, tag="sffb")
            nc.vector.tensor_scalar(
                out=sffb[:, :nb], in0=s12[:, :nb], scalar1=-1.0, scalar2=1.0,
                op0=OP.mult, op1=OP.add,
            )

            # fold sff into the one-hot weights: mp[:, b, :] = sff[:, b] * mb
            mp = mpool.tile([128, K, 64], bf16, tag="mp")
            nc.vector.tensor_mul(
                out=mp[:, :nb, :],
                in0=mb_sb[:, :nb, :],
                in1=sffb[:, :nb].unsqueeze(2).broadcast_to([128, nb, 64]),
            )

            po = psum.tile([128, 96], f32, tag="po")
            for b in range(nb):
                lo = 64 * (b // 9)
                nc.tensor.matmul(
                    out=po[lo : lo + 64, :], lhsT=mp[:, b, :], rhs=A[:, b, :],
                    start=(b % 9 == 0), stop=False,
                )
                nc.tensor.matmul(
                    out=po[lo : lo + 64, :], lhsT=mb_sb[:, b, :],
                    rhs=gt[:, b, 96:192], start=False, stop=False,
                )
                nc.tensor.matmul(
                    out=po[lo : lo + 64, :], lhsT=mb_sb[:, b, :],
                    rhs=gt[:, nb + b, 96:192], start=False, stop=(b % 9 == 8),
                )

            ot = outp.tile([128, 96], f32, tag="ot")
            nc.scalar.activation(out=ot[:nr, :], in_=po[:nr, :], func=AF.Copy)
            nc.sync.dma_start(out=out_d[n0 : n0 + nr, :], in_=ot[:nr, :])

    nc.compile()
    return nc


def _get_nc():
    global _nc_cache
    if _nc_cache is None:
        _nc_cache = _build_nc()
    return _nc_cache


def _build_idx(idx_i, idx_j):
    """idx_i/idx_j: [N, K] int -> [128, NCH, MAXCOLS] int16 wrapped layout."""
    chunks = []
    for c in range(NCH):
        n0 = c * 128
        n1 = min(n0 + 128, N)
        comb = np.concatenate(
            [idx_i[n0:n1].reshape(-1), idx_j[n0:n1].reshape(-1)]
        ).astype(np.int16)
        w = comb.reshape(-1, 16).T  # [16, ncols]; index g at (g%16, g//16)
        full = np.tile(w, (8, 1))  # replicate across the 8 q7 cores
        if full.shape[1] < MAXCOLS:
            full = np.pad(full, ((0, 0), (0, MAXCOLS - full.shape[1])))
        chunks.append(full)
    return np.ascontiguousarray(np.stack(chunks, axis=1))


def _mbase():
    """One-hot pair->node maps [128, 2K, 64] bf16; block b uses pattern b%9."""
    p = np.arange(128)[:, None, None]
    bb = (np.arange(2 * K) % 9)[None, :, None]
    m = np.arange(64)[None, None, :]
    oh = ((bb * 128 + p) // K == m).astype(ml_dtypes.bfloat16)
    return np.ascontiguousarray(oh)


def _build_table(xs, xps):
    """xs, xps: [C, N] f32 -> [N, ROW] bf16 gather table."""
    mu = xs.mean(axis=0)  # [N]
    var = xs.var(axis=0)
    t = np.zeros((N, ROW), dtype=ml_dtypes.bfloat16)
    t[:, 0:96] = (xs - mu).T
    t[:, 96:192] = xps.T
    t[:, 192] = mu
    t[:, 193] = mu * mu
    t[:, 194] = var
    return t


def _build_in_maps(x, x_p, edge_index):
    xs = np.ascontiguousarray(x[..., 0], dtype=np.float32)  # [B, C, N]
    xps = np.ascontiguousarray(x_p[..., 0], dtype=np.float32)
    idx_j_all = np.asarray(edge_index[0])  # neighbors
    idx_i_all = np.asarray(edge_index[1])  # centers
    mb = _mbase()
    return [
        {
            "table": _build_table(xs[b], xps[b]),
            "idx": _build_idx(idx_i_all[b], idx_j_all[b]),
            "mb": mb,
        }
        for b in range(B)
    ]


def kernel(x, x_p, edge_index):
    from concourse.bass_utils import run_bass_kernel_spmd

    in_maps = _build_in_maps(x, x_p, edge_index)
    nc = _get_nc()
    res = run_bass_kernel_spmd(nc, in_maps, list(range(B))).results
    out = np.stack([r["out"].T for r in res])  # [B, C, N]
    return np.ascontiguousarray(out[..., None]).astype(np.float32)


if __name__ == "__main__":
    # quick smoke test with random data
    rng = np.random.default_rng(0)
    x = rng.standard_normal((B, C, N, 1), dtype=np.float32)
    x_p = rng.random((B, C, N, 1), dtype=np.float32)
    ei = rng.integers(0, N, size=(2, B, N, K)).astype(np.int32)
    out = kernel(x, x_p, ei)
    print(out.shape, out.dtype)
